# revision 24
# baseline (speedup 1.0000x reference)
"""CCALoss (soft-contrastive CLIP + masked BCE + concept-sim KL) on 8 trn2 cores.

Math: with c = relu(mc) binary, jaccard inter = c@cT (PE matmul), union =
r_i + r_j - inter (PE computes U = r_j - inter via negated weights + a K=1
broadcast matmul of the row-sum vector). targets Tn = softmax(5*sim) row-
wise, computed as exp(5*sim - lse) so no reciprocal of Z is needed. All
three KL terms decompose into per-row dot products sum_j Tn*X plus row
lse's; a final fp32 matmul with indicator columns partition-sums the per-
row stats into [4,16] partials per core; host linearly combines 8 cores.

Data-parallel over batch rows: core k gets rows [64k, 64k+64) of the three
[512,512] logit matrices (img/txt stacked into one [128,512] tile) plus a
replicated bf16-packed transpose of medical_concepts for the jaccard
matmul.

Engine split: PE 8 matmuls; ACT groups exp ops -> ln ops -> second exp
(3 act-table loads); DVE does reductions, the union reciprocal, and BCE
elementwise chain; no gpsimd (its per-op drain cost dominates).
"""

import os
import numpy as np
from contextlib import ExitStack

import ml_dtypes

import concourse.bacc as bacc
import concourse.mybir as mybir
import concourse.tile as tile
from concourse.tile_rust import add_dep_helper
from concourse import bass_utils

F32 = mybir.dt.float32
BF16 = mybir.dt.bfloat16
AF = mybir.ActivationFunctionType
ALU = mybir.AluOpType
AX = mybir.AxisListType

B = 512          # batch
C = 256          # concepts
NCORES = 8
BLK = B // NCORES  # 64 rows per core
NST = 16         # stat columns in V (6 used, padded)

# V column layout ([128, NST]; rows 0:64 and 64:128 hold different stats)
COL_DOT_P = 0    # lower: img dot, upper: txt dot
COL_DOT_Q = 1    # lower: sim dot (H num), upper: cis dot
COL_LSE_P = 2    # lower: lse_img, upper: lse_txt
COL_LSE_Q = 3    # lower: lse_sim, upper: lse_cis
COL_BCE = 4      # lower only: sum_j mask*ln(1+e^x) (from Ln accum_out)
COL_XT = 6       # lower only: sum_j x*t
COL_MASK = 5     # lower only

_CACHE = {}


def build_nc():
    nc = bacc.Bacc(
        "TRN2", target_bir_lowering=False, debug=False, num_devices=NCORES
    )
    # [128,512] bf16: rows 0:64 = logits_per_image block, 64:128 = logits_per_text block
    pt_in = nc.dram_tensor("pt", [128, B], BF16, kind="ExternalInput").ap()
    # [64,512] bf16: concepts_image_similarity block
    cis_in = nc.dram_tensor("cis", [BLK, B], BF16, kind="ExternalInput").ap()
    # [64,512] f32: cols 0:256 concepts_logits block, cols 256:512 medical_concepts block
    clmc_in = nc.dram_tensor("clmc", [BLK, 2 * C], F32, kind="ExternalInput").ap()
    # bf16 pack: cols 0:1024 = mc^T full ([p, two*512+j] = mc[j, two*128+p]);
    # cols 1024:1152 = mc^T block cols ([p, 1024 + two*64+m] = mc[blk_m, two*128+p])
    cpack_in = nc.dram_tensor("cpack", [128, 2 * B + 2 * BLK], BF16, kind="ExternalInput").ap()
    partials = nc.dram_tensor("partials", [2, NST], F32, kind="ExternalOutput").ap()

    with tile.TileContext(nc) as tc, ExitStack() as ctx:
        pool = ctx.enter_context(tc.tile_pool(name="main", bufs=1))
        psum = ctx.enter_context(tc.tile_pool(name="psum", bufs=1, space="PSUM"))

        cp = pool.tile([128, 2 * B + 2 * BLK], BF16)   # raw mc pack
        onemc = pool.tile([128, 2 * BLK], BF16)        # 1 - relu(mc blk)
        PQ = pool.tile([128, 2 * B], BF16)  # cols 0:512 [img; txt], 512:1024 [sim5; cis]
        clmc = pool.tile([BLK, 2 * C], F32)
        V = pool.tile([128, NST], F32)

        nc.sync.dma_start(cp[:, 0:B], cpack_in[:, 0:B])
        nc.sync.dma_start(cp[:, B : 2 * B], cpack_in[:, B : 2 * B])
        nc.sync.dma_start(cp[:, 2 * B : 2 * B + 2 * BLK], cpack_in[:, 2 * B : 2 * B + 2 * BLK])
        nc.sync.dma_start(clmc[:], clmc_in[:])
        nc.sync.dma_start(PQ[:, 0:B], pt_in[:])
        nc.sync.dma_start(PQ[64:128, B : 2 * B], cis_in[:])

        nc.gpsimd.memset(V[:], 0.0)

        # c = relu(mc): -1 (missing) -> 0; split so matmuls start per-chunk
        nc.vector.tensor_scalar_max(cp[:, 0:B], cp[:, 0:B], 0.0)
        nc.vector.tensor_scalar_max(cp[:, B : 2 * B], cp[:, B : 2 * B], 0.0)
        # onemc = 1 - relu(mc) == (mc <= 0), computed straight from raw values
        nc.vector.tensor_scalar(
            onemc[:], cp[:, 2 * B : 2 * B + 2 * BLK], 0.0, None, ALU.is_le
        )
        nc.vector.tensor_scalar(
            cp[:, 2 * B : 2 * B + 2 * BLK], cp[:, 2 * B : 2 * B + 2 * BLK],
            0.0, None, ALU.max,
        )
        cfull = cp[:, 0 : 2 * B]
        cblk = cp[:, 2 * B : 2 * B + 2 * BLK]

        # --- BCE block: bce = ln(1 + e^x) - x*t, masked ---
        cl_s = clmc[:, 0:C]
        mc_s = clmc[:, C : 2 * C]
        tpos = pool.tile([BLK, C], F32)
        tmask = pool.tile([BLK, C], F32)
        nc.vector.tensor_scalar_max(tpos[:], mc_s, 0.0)
        nc.vector.tensor_scalar(tmask[:], mc_s, -1.0, None, ALU.not_equal)
        r_blk = pool.tile([BLK, 1], F32)
        nc.vector.reduce_sum(r_blk[:], tpos[:], axis=AX.X)

        bexp = pool.tile([BLK, C], F32)
        i_bexp = nc.scalar.activation(bexp[:], cl_s, AF.Exp).ins  # e^x (x ~ N(0,1))

        # --- jaccard via matmul ---
        # r_j - inter[i,j] = sum_k (1 - c_ik) * c_jk: U in ONE matmul pass
        p_U = psum.tile([BLK, B], F32)
        nc.tensor.matmul(p_U[:], onemc[:, 0:BLK], cfull[:, 0:B], start=True, stop=False)
        nc.tensor.matmul(p_U[:], onemc[:, BLK : 2 * BLK], cfull[:, B : 2 * B], start=False, stop=True)

        p_inter = psum.tile([BLK, B], F32)
        nc.tensor.matmul(p_inter[:], cblk[:, 0:BLK], cfull[:, 0:B], start=True, stop=False)
        nc.tensor.matmul(p_inter[:], cblk[:, BLK : 2 * BLK], cfull[:, B : 2 * B], start=False, stop=True)

        # u = max(r_i + (r_j - inter), 0.5); exact integers when > 0
        u = pool.tile([BLK, B], F32)
        nc.vector.tensor_scalar(u[:], p_U[:], r_blk[:], 0.5, ALU.add, ALU.max)
        urec = pool.tile([BLK, B], F32)
        # ~51 ULP approx is plenty: u in [0.5, 512]; error ~4e-6 relative
        nc.vector.reciprocal_approx_fast(urec[:], u[:])
        # 5*inter via ACT copy from psum (Copy lives in every act table)
        inter5 = pool.tile([BLK, B], F32)
        nc.scalar.activation(inter5[:], p_inter[:], AF.Copy, scale=5.0)
        nc.vector.tensor_tensor(PQ[0:BLK, B : 2 * B], inter5[:], urec[:], ALU.mult)  # sim5

        # BCE-front elementwise work backfills the DVE idle window here
        # premask: ln(1 + mask*e^x) == mask * ln(1 + e^x) exactly (mask in {0,1})
        nc.vector.tensor_tensor(bexp[:], bexp[:], tmask[:], ALU.mult)
        b2 = pool.tile([BLK, C], F32)
        nc.vector.tensor_tensor(b2[:], cl_s, tpos[:], ALU.mult)
        nc.vector.reduce_sum(V[0:BLK, COL_XT : COL_XT + 1], b2[:], axis=AX.X)
        nc.vector.reduce_sum(V[0:BLK, COL_MASK : COL_MASK + 1], tmask[:], axis=AX.X)

        # --- softmax stats over Q = [sim5; cis] and PT = [img; txt] ---
        # inputs are N(0,1) logits and sim5 in [0,5]: exp never overflows fp32,
        # so skip the max-subtraction entirely; lse_i = ln Z_i directly.
        eQ = pool.tile([128, B], F32)
        ZQ = pool.tile([128, 1], F32)
        nc.scalar.activation(
            eQ[BLK:128, :], PQ[BLK:128, B : 2 * B], AF.Exp,
            accum_out=ZQ[BLK:128, :],
        )
        i_eQ = nc.scalar.activation(
            eQ[0:BLK, :], PQ[0:BLK, B : 2 * B], AF.Exp,
            accum_out=ZQ[0:BLK, :],
        ).ins

        eP = pool.tile([128, B], F32, tag="escr")
        ZP = pool.tile([128, 1], F32)
        i_eP = nc.scalar.activation(eP[:], PQ[:, 0:B], AF.Exp, accum_out=ZP[:]).ins

        bln = pool.tile([BLK, C], F32)
        # masked softplus summed by the ACT accumulator: no DVE tail at all
        i_bln = nc.scalar.activation(
            bln[:], bexp[:], AF.Ln, bias=1.0,
            accum_out=V[0:BLK, COL_BCE : COL_BCE + 1],
        ).ins
        # Ln writes the lse V-columns directly (lse = ln Z, no max to add back)
        i_lnZQ = nc.scalar.activation(V[:, COL_LSE_Q : COL_LSE_Q + 1], ZQ[:], AF.Ln).ins
        i_lnZP = nc.scalar.activation(V[:, COL_LSE_P : COL_LSE_P + 1], ZP[:], AF.Ln).ins
        # keep ACT ops grouped exp -> ln so only 2 act-table loads occur
        add_dep_helper(i_bln, i_eQ, False, "act-table-group")
        add_dep_helper(i_bln, i_eP, False, "act-table-group")
        add_dep_helper(i_lnZP, i_eQ, False, "act-table-group")

        # --- Tn = e_sim / Z_sim via approx reciprocal, duplicated into both halves ---
        zrec = pool.tile([BLK, 1], F32)
        nc.vector.reciprocal_approx_fast(zrec[:], ZQ[0:BLK, :])
        TnD = pool.tile([128, B], BF16)
        nc.vector.tensor_scalar_mul(TnD[0:BLK, :], eQ[0:BLK, :], zrec[:])
        nc.vector.tensor_scalar_mul(TnD[BLK:128, :], eQ[0:BLK, :], zrec[:])

        # --- weighted dots: sum_j Tn * X for all four X in two fused ops ---
        mPQ = pool.tile([128, 2 * B], BF16)
        tn_b = TnD[:].rearrange("p (two b) -> p two b", two=1, b=B)
        tn_b = tn_b.broadcast_to((128, 2, B))  # step-0 middle dim
        pq_3d = PQ[:].rearrange("p (two b) -> p two b", two=2, b=B)
        m_3d = mPQ[:].rearrange("p (two b) -> p two b", two=2, b=B)
        nc.vector.tensor_tensor(m_3d, tn_b, pq_3d, ALU.mult)
        nc.vector.reduce_sum(V[:, COL_DOT_P : COL_DOT_Q + 1], m_3d, axis=AX.X)

        # --- partition-sum via fp32 matmul with indicator columns ---
        ind = pool.tile([128, 2], F32)
        nc.vector.memset(ind[:], 0.0)
        nc.vector.memset(ind[0:BLK, 0:1], 1.0)
        nc.vector.memset(ind[BLK:128, 1:2], 1.0)
        p_out = psum.tile([2, NST], F32)
        nc.tensor.matmul(p_out[:], ind[:], V[:], start=True, stop=True)
        out_sb = pool.tile([2, NST], F32)
        nc.vector.tensor_copy(out_sb[:], p_out[:])
        nc.sync.dma_start(partials[:], out_sb[:])

    nc.compile()
    return nc


def _pack_T(mc_cols: np.ndarray) -> np.ndarray:
    """[256, W] bf16 -> [128, 2*W] with col two*W+j on partition p = row two*128+p."""
    w = mc_cols.shape[1]
    return np.ascontiguousarray(
        mc_cols.reshape(2, 128, w).transpose(1, 0, 2).reshape(128, 2 * w)
    )


def make_in_maps(inputs):
    li = np.asarray(inputs["logits_per_image"], dtype=np.float32)
    lt = np.asarray(inputs["logits_per_text"], dtype=np.float32)
    cl = np.asarray(inputs["concepts_logits"], dtype=np.float32)
    cis = np.asarray(inputs["concepts_image_similarity"], dtype=np.float32)
    mc = np.asarray(inputs["medical_concepts"])

    mcT = np.ascontiguousarray(mc.T).astype(ml_dtypes.bfloat16)  # [256, 512]
    full_pack = _pack_T(mcT)  # [128, 1024]
    in_maps = []
    for k in range(NCORES):
        sl = slice(k * BLK, (k + 1) * BLK)
        blk_pack = _pack_T(np.ascontiguousarray(mcT[:, sl]))  # [128, 128]
        cpack = np.concatenate([full_pack, blk_pack], axis=1)  # [128, 1152]
        in_maps.append({
            "pt": np.concatenate([li[sl], lt[sl]], axis=0).astype(ml_dtypes.bfloat16),
            "cis": np.ascontiguousarray(cis[sl]).astype(ml_dtypes.bfloat16),
            "clmc": np.concatenate(
                [cl[sl], mc[sl].astype(np.float32)], axis=1),         # [64, 512]
            "cpack": np.ascontiguousarray(cpack),
        })
    return in_maps


def combine_partials(parts) -> np.ndarray:
    s = np.sum(np.stack(parts, 0).astype(np.float64), axis=0)  # [4, NST]
    dot_pt = s[0, COL_DOT_P] + s[1, COL_DOT_P]      # img + txt numerators
    dot_h = s[0, COL_DOT_Q]                          # sim (H) numerator
    dot_cis = s[1, COL_DOT_Q]
    lse_pt = s[0, COL_LSE_P] + s[1, COL_LSE_P]
    lse_sim = s[0, COL_LSE_Q]
    lse_cis = s[1, COL_LSE_Q]
    bce_sum = s[0, COL_BCE] - s[0, COL_XT]
    mask_sum = s[0, COL_MASK]

    H = dot_h - lse_sim                 # sum_i (sum_j T log T)
    a_pt = dot_pt - lse_pt              # sum_i (A_img + A_txt)
    a_cis = dot_cis - lse_cis
    clip = (2.0 * H - a_pt) / (2.0 * B)
    csim = (H - a_cis) / B
    conc = bce_sum / (mask_sum + 1e-8)
    total = clip + 0.2 * conc + 0.2 * csim
    return np.asarray(total, dtype=np.float32)


def _run(inputs, trace=False):
    if "nc" not in _CACHE:
        _CACHE["nc"] = build_nc()
    nc = _CACHE["nc"]
    res = bass_utils.run_bass_kernel_spmd(
        nc, make_in_maps(inputs), core_ids=list(range(NCORES)), trace=trace
    )
    parts = [res.results[k]["partials"] for k in range(NCORES)]
    return combine_partials(parts), res


def kernel(**inputs) -> np.ndarray:
    out, _ = _run(inputs, trace=bool(int(os.environ.get("KERNEL_TRACE", "0"))))
    return out


# revision 27
# speedup vs baseline: 1.0896x; 1.0896x over previous
"""CCALoss (soft-contrastive CLIP + masked BCE + concept-sim KL) on 8 trn2 cores.

Math: with c = relu(mc) binary, jaccard inter = c@cT (PE matmul), union =
r_i + r_j - inter (PE computes U = r_j - inter via negated weights + a K=1
broadcast matmul of the row-sum vector). targets Tn = softmax(5*sim) row-
wise, computed as exp(5*sim - lse) so no reciprocal of Z is needed. All
three KL terms decompose into per-row dot products sum_j Tn*X plus row
lse's; a final fp32 matmul with indicator columns partition-sums the per-
row stats into [4,16] partials per core; host linearly combines 8 cores.

Data-parallel over batch rows: core k gets rows [64k, 64k+64) of the three
[512,512] logit matrices (img/txt stacked into one [128,512] tile) plus a
replicated bf16-packed transpose of medical_concepts for the jaccard
matmul.

Engine split: PE 8 matmuls; ACT groups exp ops -> ln ops -> second exp
(3 act-table loads); DVE does reductions, the union reciprocal, and BCE
elementwise chain; no gpsimd (its per-op drain cost dominates).
"""

import os
import numpy as np
from contextlib import ExitStack

import ml_dtypes

import concourse.bacc as bacc
import concourse.mybir as mybir
import concourse.tile as tile
from concourse.tile_rust import add_dep_helper
from concourse import bass_utils

F32 = mybir.dt.float32
BF16 = mybir.dt.bfloat16
AF = mybir.ActivationFunctionType
ALU = mybir.AluOpType
AX = mybir.AxisListType

B = 512          # batch
C = 256          # concepts
NCORES = 8
BLK = B // NCORES  # 64 rows per core
NST = 16         # stat columns in V (6 used, padded)

# V column layout ([128, NST]; rows 0:64 and 64:128 hold different stats)
COL_DOT_P = 0    # lower: img dot, upper: txt dot
COL_DOT_Q = 1    # lower: sim dot (H num), upper: cis dot
COL_LSE_P = 2    # lower: lse_img, upper: lse_txt
COL_LSE_Q = 3    # lower: lse_sim, upper: lse_cis
COL_BCE = 4      # lower only: sum_j mask*ln(1+e^x) (from Ln accum_out)
COL_XT = 6       # lower only: sum_j x*t
COL_MASK = 5     # lower only

_CACHE = {}


def build_nc():
    nc = bacc.Bacc(
        "TRN2", target_bir_lowering=False, debug=False, num_devices=NCORES
    )
    # [128,512] f32: rows 0:64 = logits_per_image block, 64:128 = logits_per_text block
    pt_in = nc.dram_tensor("pt", [128, B], F32, kind="ExternalInput").ap()
    # [64,512] f32: concepts_image_similarity block
    cis_in = nc.dram_tensor("cis", [BLK, B], F32, kind="ExternalInput").ap()
    # [64,512] f32: cols 0:256 concepts_logits block, cols 256:512 medical_concepts block
    clmc_in = nc.dram_tensor("clmc", [BLK, 2 * C], F32, kind="ExternalInput").ap()
    # bf16 pack: cols 0:1024 = mc^T full ([p, two*512+j] = mc[j, two*128+p]);
    # cols 1024:1152 = mc^T block cols ([p, 1024 + two*64+m] = mc[blk_m, two*128+p])
    cpack_in = nc.dram_tensor("cpack", [128, 2 * B + 2 * BLK], BF16, kind="ExternalInput").ap()
    partials = nc.dram_tensor("partials", [4, NST], F32, kind="ExternalOutput").ap()

    with tile.TileContext(nc) as tc, ExitStack() as ctx:
        pool = ctx.enter_context(tc.tile_pool(name="main", bufs=1))
        psum = ctx.enter_context(tc.tile_pool(name="psum", bufs=1, space="PSUM"))

        cp = pool.tile([128, 2 * B + 2 * BLK], BF16)   # raw mc pack
        onemc = pool.tile([128, 2 * BLK], BF16)        # 1 - relu(mc blk)
        PQ = pool.tile([128, 2 * B], F32)  # cols 0:512 [img; txt], 512:1024 [sim5; cis]
        clmc = pool.tile([BLK, 2 * C], F32)
        V = pool.tile([128, NST], F32)

        nc.sync.dma_start(cp[:, 0:B], cpack_in[:, 0:B])
        nc.sync.dma_start(cp[:, B : 2 * B], cpack_in[:, B : 2 * B])
        nc.sync.dma_start(cp[:, 2 * B : 2 * B + 2 * BLK], cpack_in[:, 2 * B : 2 * B + 2 * BLK])
        nc.sync.dma_start(clmc[:], clmc_in[:])
        nc.sync.dma_start(PQ[:, 0:B], pt_in[:])
        nc.sync.dma_start(PQ[64:128, B : 2 * B], cis_in[:])

        nc.gpsimd.memset(V[:], 0.0)

        # c = relu(mc): -1 (missing) -> 0; split so matmuls start per-chunk
        nc.vector.tensor_scalar_max(cp[:, 0:B], cp[:, 0:B], 0.0)
        nc.vector.tensor_scalar_max(cp[:, B : 2 * B], cp[:, B : 2 * B], 0.0)
        # onemc = 1 - relu(mc) == (mc <= 0), computed straight from raw values
        nc.vector.tensor_scalar(
            onemc[:], cp[:, 2 * B : 2 * B + 2 * BLK], 0.0, None, ALU.is_le
        )
        nc.vector.tensor_scalar(
            cp[:, 2 * B : 2 * B + 2 * BLK], cp[:, 2 * B : 2 * B + 2 * BLK],
            0.0, None, ALU.max,
        )
        cfull = cp[:, 0 : 2 * B]
        cblk = cp[:, 2 * B : 2 * B + 2 * BLK]

        # --- BCE block: bce = ln(1 + e^x) - x*t, masked ---
        cl_s = clmc[:, 0:C]
        mc_s = clmc[:, C : 2 * C]
        tpos = pool.tile([BLK, C], F32)
        tmask = pool.tile([BLK, C], F32)
        nc.vector.tensor_scalar_max(tpos[:], mc_s, 0.0)
        nc.vector.tensor_scalar(tmask[:], mc_s, -1.0, None, ALU.not_equal)
        r_blk = pool.tile([BLK, 1], F32)
        nc.vector.reduce_sum(r_blk[:], tpos[:], axis=AX.X)

        bexp = pool.tile([BLK, C], F32)
        i_bexp = nc.scalar.activation(bexp[:], cl_s, AF.Exp).ins  # e^x (x ~ N(0,1))

        # --- jaccard via matmul ---
        # r_j - inter[i,j] = sum_k (1 - c_ik) * c_jk: U in ONE matmul pass
        p_U = psum.tile([BLK, B], F32)
        nc.tensor.matmul(p_U[:], onemc[:, 0:BLK], cfull[:, 0:B], start=True, stop=False)
        nc.tensor.matmul(p_U[:], onemc[:, BLK : 2 * BLK], cfull[:, B : 2 * B], start=False, stop=True)

        p_inter = psum.tile([BLK, B], F32)
        nc.tensor.matmul(p_inter[:], cblk[:, 0:BLK], cfull[:, 0:B], start=True, stop=False)
        nc.tensor.matmul(p_inter[:], cblk[:, BLK : 2 * BLK], cfull[:, B : 2 * B], start=False, stop=True)

        # u = max(r_i + (r_j - inter), 0.5); exact integers when > 0
        u = pool.tile([BLK, B], F32)
        nc.vector.tensor_scalar(u[:], p_U[:], r_blk[:], 0.5, ALU.add, ALU.max)
        urec = pool.tile([BLK, B], F32)
        # ~51 ULP approx is plenty: u in [0.5, 512]; error ~4e-6 relative
        nc.vector.reciprocal_approx_fast(urec[:], u[:])
        # 5*inter via ACT copy from psum (Copy lives in every act table)
        inter5 = pool.tile([BLK, B], F32)
        nc.scalar.activation(inter5[:], p_inter[:], AF.Copy, scale=5.0)
        nc.vector.tensor_tensor(PQ[0:BLK, B : 2 * B], inter5[:], urec[:], ALU.mult)  # sim5

        # BCE-front elementwise work backfills the DVE idle window here
        # premask: ln(1 + mask*e^x) == mask * ln(1 + e^x) exactly (mask in {0,1})
        nc.vector.tensor_tensor(bexp[:], bexp[:], tmask[:], ALU.mult)
        b2 = pool.tile([BLK, C], F32)
        nc.vector.tensor_tensor(b2[:], cl_s, tpos[:], ALU.mult)
        nc.vector.reduce_sum(V[0:BLK, COL_XT : COL_XT + 1], b2[:], axis=AX.X)
        nc.vector.reduce_sum(V[0:BLK, COL_MASK : COL_MASK + 1], tmask[:], axis=AX.X)

        # --- softmax stats over Q = [sim5; cis] and PT = [img; txt] ---
        # inputs are N(0,1) logits and sim5 in [0,5]: exp never overflows fp32,
        # so skip the max-subtraction entirely; lse_i = ln Z_i directly.
        eD = pool.tile([128, B], F32)
        ZQ = pool.tile([128, 1], F32)
        ecis = pool.tile([BLK, B], F32, tag="escr")
        nc.scalar.activation(
            ecis[:], PQ[BLK:128, B : 2 * B], AF.Exp,
            accum_out=ZQ[BLK:128, :],
        )
        i_eQ = nc.scalar.activation(
            eD[0:BLK, :], PQ[0:BLK, B : 2 * B], AF.Exp,
            accum_out=ZQ[0:BLK, :],
        ).ins

        eP = pool.tile([128, B], F32, tag="escr")
        ZP = pool.tile([128, 1], F32)
        i_eP = nc.scalar.activation(eP[:], PQ[:, 0:B], AF.Exp, accum_out=ZP[:]).ins

        bln = pool.tile([BLK, C], F32)
        # masked softplus summed by the ACT accumulator: no DVE tail at all
        i_bln = nc.scalar.activation(
            bln[:], bexp[:], AF.Ln, bias=1.0,
            accum_out=V[0:BLK, COL_BCE : COL_BCE + 1],
        ).ins
        # Ln writes the lse V-columns directly (lse = ln Z, no max to add back)
        i_lnZQ = nc.scalar.activation(V[:, COL_LSE_Q : COL_LSE_Q + 1], ZQ[:], AF.Ln).ins
        i_lnZP = nc.scalar.activation(V[:, COL_LSE_P : COL_LSE_P + 1], ZP[:], AF.Ln).ins
        # keep ACT ops grouped exp -> ln so only 2 act-table loads occur
        add_dep_helper(i_bln, i_eQ, False, "act-table-group")
        add_dep_helper(i_bln, i_eP, False, "act-table-group")
        add_dep_helper(i_lnZP, i_eQ, False, "act-table-group")

        # --- raw-e dots; 1/Z normalization happens inside the final matmul ---
        # duplicate e_sim into the upper partition half (one copy, no Tn pass)
        nc.vector.tensor_copy(eD[BLK:128, :], eD[0:BLK, :])

        mPQ = pool.tile([128, 2 * B], F32)
        e_b = eD[:].rearrange("p (two b) -> p two b", two=1, b=B)
        e_b = e_b.broadcast_to((128, 2, B))
        pq_3d = PQ[:].rearrange("p (two b) -> p two b", two=2, b=B)
        m_3d = mPQ[:].rearrange("p (two b) -> p two b", two=2, b=B)
        nc.vector.tensor_tensor(m_3d, e_b, pq_3d, ALU.mult)
        nc.vector.reduce_sum(V[:, COL_DOT_P : COL_DOT_Q + 1], m_3d, axis=AX.X)

        # --- partition-sum matmul: rows 0/1 weight dots by 1/Z_sim, rows 2/3 plain sums ---
        zrec = pool.tile([BLK, 1], F32)
        nc.vector.reciprocal_approx_fast(zrec[:], ZQ[0:BLK, :])
        ind = pool.tile([128, 4], F32)
        nc.vector.memset(ind[:], 0.0)
        nc.vector.tensor_copy(ind[0:BLK, 0:1], zrec[:])
        nc.vector.tensor_copy(ind[BLK:128, 1:2], zrec[:])
        nc.vector.memset(ind[0:BLK, 2:3], 1.0)
        nc.vector.memset(ind[BLK:128, 3:4], 1.0)
        p_out = psum.tile([4, NST], F32)
        nc.tensor.matmul(p_out[:], ind[:], V[:], start=True, stop=True)
        out_sb = pool.tile([4, NST], F32)
        nc.vector.tensor_copy(out_sb[:], p_out[:])
        nc.sync.dma_start(partials[:], out_sb[:])

    nc.compile()
    return nc


def _pack_T(mc_cols: np.ndarray) -> np.ndarray:
    """[256, W] bf16 -> [128, 2*W] with col two*W+j on partition p = row two*128+p."""
    w = mc_cols.shape[1]
    return np.ascontiguousarray(
        mc_cols.reshape(2, 128, w).transpose(1, 0, 2).reshape(128, 2 * w)
    )


def make_in_maps(inputs):
    li = np.asarray(inputs["logits_per_image"], dtype=np.float32)
    lt = np.asarray(inputs["logits_per_text"], dtype=np.float32)
    cl = np.asarray(inputs["concepts_logits"], dtype=np.float32)
    cis = np.asarray(inputs["concepts_image_similarity"], dtype=np.float32)
    mc = np.asarray(inputs["medical_concepts"])

    mcT = np.ascontiguousarray(mc.T).astype(ml_dtypes.bfloat16)  # [256, 512]
    full_pack = _pack_T(mcT)  # [128, 1024]
    in_maps = []
    for k in range(NCORES):
        sl = slice(k * BLK, (k + 1) * BLK)
        blk_pack = _pack_T(np.ascontiguousarray(mcT[:, sl]))  # [128, 128]
        cpack = np.concatenate([full_pack, blk_pack], axis=1)  # [128, 1152]
        in_maps.append({
            "pt": np.concatenate([li[sl], lt[sl]], axis=0),          # [128, 512]
            "cis": np.ascontiguousarray(cis[sl]),                     # [64, 512]
            "clmc": np.concatenate(
                [cl[sl], mc[sl].astype(np.float32)], axis=1),         # [64, 512]
            "cpack": np.ascontiguousarray(cpack),
        })
    return in_maps


def combine_partials(parts) -> np.ndarray:
    s = np.sum(np.stack(parts, 0).astype(np.float64), axis=0)  # [4, NST]
    # rows 0/1: 1/Z_sim-weighted partition sums (dots); rows 2/3: plain sums
    dot_pt = s[0, COL_DOT_P] + s[1, COL_DOT_P]      # img + txt numerators
    dot_h = s[0, COL_DOT_Q]                          # sim (H) numerator
    dot_cis = s[1, COL_DOT_Q]
    lse_pt = s[2, COL_LSE_P] + s[3, COL_LSE_P]
    lse_sim = s[2, COL_LSE_Q]
    lse_cis = s[3, COL_LSE_Q]
    bce_sum = s[2, COL_BCE] - s[2, COL_XT]
    mask_sum = s[2, COL_MASK]

    H = dot_h - lse_sim                 # sum_i (sum_j T log T)
    a_pt = dot_pt - lse_pt              # sum_i (A_img + A_txt)
    a_cis = dot_cis - lse_cis
    clip = (2.0 * H - a_pt) / (2.0 * B)
    csim = (H - a_cis) / B
    conc = bce_sum / (mask_sum + 1e-8)
    total = clip + 0.2 * conc + 0.2 * csim
    return np.asarray(total, dtype=np.float32)


def _run(inputs, trace=False):
    if "nc" not in _CACHE:
        _CACHE["nc"] = build_nc()
    nc = _CACHE["nc"]
    res = bass_utils.run_bass_kernel_spmd(
        nc, make_in_maps(inputs), core_ids=list(range(NCORES)), trace=trace
    )
    parts = [res.results[k]["partials"] for k in range(NCORES)]
    return combine_partials(parts), res


def kernel(**inputs) -> np.ndarray:
    out, _ = _run(inputs, trace=bool(int(os.environ.get("KERNEL_TRACE", "0"))))
    return out
